# revision 5
# baseline (speedup 1.0000x reference)
"""LinearSelfAttention (elu+1 linear attention) Trainium2 Bass kernel, v3.

Full inputs -> full output. Shards the 32768 tokens (B=4 x N=8192) across 8
NeuronCores as (batch, seq-half); the small kv / k-sum statistics are
all-reduced between the two cores sharing a batch. Weights are replicated.

Per-core structure (T=4096 tokens, 8 chunks of 512):
  - x arrives host-pre-transposed as x'^T [512 feat, 4096 tok] so
    feature-major tiles DMA straight into SBUF (no PE transposes).
  - Weights host-packed to SBUF layout; startup DMAs split per-kc and
    interleaved with the first x chunk so the first matmul starts ~3us in.
  phase 1: k,v projections (fp32r, token-major), elu+1(k), per head-pair
    kv/ksum accumulation [k_2c|k_2c+1].T @ [v_2c |1| v_2c+1 |1] (N=130).
    The accum matmuls for chunk ci are emitted after chunk ci+1's k/v
    matmuls (software pipelining hides the elu ACT/DVE latency).
  AllReduce of tight-packed stats [128,4,65] between seq-half pairs.
  q' projection (+elu) runs after the collective is triggered and hides
    its latency.
  phase 2: out'[e,n] via block-diag kv lhsT; denominator via ksum-broadcast
    block-diag lhsT; z = exp(-ln(den)) on ACT (den is large positive);
    ost = out'*z (DVE); y = ost.T @ Wout + bias (bias via DVE add on the
    PSUM->SBUF move). y matmuls for chunk ci emitted after chunk ci+1's
    out/den matmuls (pipelining); last chunk's store split per t-tile.

All matmul operands are float32r (fp22-rounded fp32) - full PE rate for
N>=256, ~1e-4 relative error. fp32r matmuls require even N and outputs at
base partition 0. All PSUM->SBUF moves/elementwise run explicitly on DVE
(nc.vector) except transcendentals (ACT) - nc.any routes big copies to the
much slower ACT path.
"""

import numpy as np

import concourse.bass as bass
import concourse.bacc as bacc
import concourse.mybir as mybir
import concourse.tile as tile
from concourse.bass_utils import run_bass_kernel_spmd

B, N, D, H, HD = 4, 8192, 512, 8, 64
NCORES = 8
T = B * N // NCORES          # 4096 tokens per core
NT = 512                     # tokens per chunk
NCH = T // NT                # 8 chunks
VW = 2 * HD + 2              # 130: [v_2c | 1 | v_2c+1 | 1]
SW = HD + 1                  # 65: [kv_h | ksum_h] packed width
F32 = mybir.dt.float32
F32R = mybir.dt.float32r
AF = mybir.ActivationFunctionType
OP = mybir.AluOpType

REPLICA_GROUPS = [[0, 1], [2, 3], [4, 5], [6, 7]]


def _r(ap):
    return ap.bitcast(F32R)


def _build_kernel(tc, nc, xt_d, wqkv_d, wout_d, bias_d, y_d):
    with (
        tc.tile_pool(name="consts", bufs=1) as consts,
        tc.tile_pool(name="persist", bufs=1) as persist,
        tc.tile_pool(name="small", bufs=2) as small,
        tc.tile_pool(name="dram", bufs=1, space="DRAM") as dram,
    ):
        xt_src = xt_d.rearrange("(kc p) t -> p kc t", p=128)

        # ---------------- weights / constants / first x chunk --------------
        # Interleave the first x chunk's t-slices with the per-kc k-weight
        # slices so the first k matmul is gated on only ~0.5 MB of DMA.
        wqkv_sb = consts.tile([128, 3, 4, D], F32R)
        with tc.tile_pool(name="p1work", bufs=2) as work, \
             tc.tile_pool(name="ps1", bufs=2, space="PSUM") as psum:
            xt0 = work.tile([128, 4, NT], F32R, tag="xt")
            for t in range(4):
                nc.sync.dma_start(
                    out=xt0[:, :, t * 128:(t + 1) * 128],
                    in_=_r(xt_src[:, :, t * 128:(t + 1) * 128]),
                )
                nc.sync.dma_start(out=wqkv_sb[:, 0, t, :], in_=_r(wqkv_d[:, 0, t, :]))
            for kc in range(4):
                nc.sync.dma_start(out=wqkv_sb[:, 1, kc, :], in_=_r(wqkv_d[:, 1, kc, :]))
            nc.sync.dma_start(out=wqkv_sb[:, 2], in_=_r(wqkv_d[:, 2]))
            wout_sb = consts.tile([128, 4, D], F32R)
            nc.sync.dma_start(out=wout_sb, in_=_r(wout_d))
            bias_sb = consts.tile([128, D], F32)
            nc.sync.dma_start(out=bias_sb, in_=bias_d)

            scr_f32 = consts.tile([128, 128], F32)
            nc.vector.memset(scr_f32, 1.0)
            ones_col = consts.tile([128, HD], F32R)
            nc.vector.tensor_copy(ones_col, scr_f32[:, 0:HD])
            ones441 = consts.tile([128, 4, 1], F32R)
            nc.vector.tensor_copy(ones441, scr_f32[:, 0:4].rearrange("p (t o) -> p t o", o=1))
            zscr_f32 = consts.tile([128, 128], F32)
            nc.vector.memset(zscr_f32, 0.0)
            zeros_sb = consts.tile([128, 128], F32R)
            nc.vector.tensor_copy(zeros_sb, zscr_f32)

            # q'+ (elu(q)+1), feature-major, persistent: [fo, n]
            qp_sb = persist.tile([128, 4, T], F32R)
            cc_sb = persist.tile([128, 4, VW], F32)
            nc.vector.memset(cc_sb, 0.0)

            # ---------------- phase 1: k,v -> kv/ksum stats ----------------
            def kv_chunk(ci, xt_sb):
                """k/v projection + elu(k) for chunk ci; returns (kp, v_sb)."""
                v_sb = work.tile([128, 4, 4 * VW], F32R, tag="vsb")
                for c in range(4):
                    for u in range(2):
                        col = c * VW + HD + u * (HD + 1)
                        nc.vector.tensor_copy(v_sb[:, :, col:col + 1], ones441)
                kp = work.tile([128, 4, D], F32R, tag="kp")
                for t in range(4):
                    k_ps = psum.tile([128, D], F32, tag="kps")
                    v_ps = psum.tile([128, D], F32, tag="vps")
                    for kc in range(4):
                        st, sp = kc == 0, kc == 3
                        lhsT = xt_sb[:, kc, t * 128:(t + 1) * 128]
                        nc.tensor.matmul(k_ps, lhsT, wqkv_sb[:, 0, kc, :], start=st, stop=sp)
                        nc.tensor.matmul(v_ps, lhsT, wqkv_sb[:, 1, kc, :], start=st, stop=sp)
                    nc.vector.tensor_copy(
                        v_sb[:, t, :].rearrange("p (c u e) -> p c u e", c=4, u=2)[:, :, :, 0:HD],
                        v_ps.rearrange("p (c u e) -> p c u e", c=4, u=2),
                    )
                    # elu(k)+1 = min(exp(k),1) + relu(k)
                    e_sb = small.tile([128, D], F32, tag="e")
                    nc.scalar.activation(e_sb, k_ps, AF.Exp)
                    r_sb = small.tile([128, D], F32, tag="r")
                    nc.scalar.activation(r_sb, k_ps, AF.Relu)
                    nc.vector.scalar_tensor_tensor(kp[:, t, :], e_sb, 1.0, r_sb, OP.min, OP.add)
                return kp, v_sb

            def kv_accum(kp, v_sb):
                for c in range(4):
                    acc_ps = psum.tile([128, VW], F32, tag="acc")
                    for t in range(4):
                        nc.tensor.matmul(
                            acc_ps,
                            kp[:, t, c * 128:(c + 1) * 128],
                            v_sb[:, t, c * VW:(c + 1) * VW],
                            start=(t == 0), stop=(t == 3),
                        )
                    nc.vector.tensor_add(cc_sb[:, c, :], cc_sb[:, c, :], acc_ps)

            prev = kv_chunk(0, xt0)
            for ci in range(1, NCH):
                xt_sb = work.tile([128, 4, NT], F32R, tag="xt")
                nc.sync.dma_start(out=xt_sb, in_=_r(xt_src[:, :, ci * NT:(ci + 1) * NT]))
                cur = kv_chunk(ci, xt_sb)
                kv_accum(*prev)  # pipelined: hides chunk ci-1's elu latency
                prev = cur
            kv_accum(*prev)

        # ---------------- all-reduce kv/ksum between seq-half pairs --------
        # tight-pack [128,4,130] -> [128,4,65]: rows 0:64 hold [kv_2c|ksum],
        # rows 64:128 hold [kv_2c+1|ksum] (halves the collective payload).
        with tc.tile_pool(name="qwork", bufs=2) as qwork, \
             tc.tile_pool(name="psq", bufs=2, space="PSUM") as psq:
            # prefetch the first two q'-pass x chunks so the PE rolls straight
            # from phase 1 into the q' projection
            xtq_pre = []
            for ci in range(2):
                xt_sb = qwork.tile([128, 4, NT], F32R, tag="xtq")
                nc.sync.dma_start(out=xt_sb, in_=_r(xt_src[:, :, ci * NT:(ci + 1) * NT]))
                xtq_pre.append(xt_sb)

            cc_tx = persist.tile([128, 4, SW], F32)
            for c in range(4):
                nc.vector.tensor_copy(cc_tx[0:64, c, :], cc_sb[0:64, c, 0:SW])
                nc.vector.tensor_copy(cc_tx[64:128, c, :], cc_sb[64:128, c, SW:2 * SW])
            cc_in = dram.tile([128, 4, SW], F32)
            cc_out = dram.tile([128, 4, SW], F32)
            nc.sync.dma_start(out=cc_in, in_=cc_tx)
            nc.gpsimd.collective_compute(
                "AllReduce", OP.add,
                replica_groups=REPLICA_GROUPS,
                ins=[cc_in.opt()], outs=[cc_out.opt()],
            )
            ar_sb = persist.tile([128, 4, SW], F32)
            nc.sync.dma_start(out=ar_sb, in_=cc_out)

            # ---------------- q' projection (overlaps the collective) -------
            for ci in range(NCH):
                if ci < 2:
                    xt_sb = xtq_pre[ci]
                else:
                    xt_sb = qwork.tile([128, 4, NT], F32R, tag="xtq")
                    nc.sync.dma_start(out=xt_sb, in_=_r(xt_src[:, :, ci * NT:(ci + 1) * NT]))
                for c in range(4):
                    q_ps = psq.tile([128, NT], F32, tag="qps")
                    for kc in range(4):
                        nc.tensor.matmul(
                            q_ps,
                            wqkv_sb[:, 2, kc, c * 128:(c + 1) * 128],
                            xt_sb[:, kc, :],
                            start=(kc == 0), stop=(kc == 3),
                        )
                    e2 = small.tile([128, NT], F32, tag="e")
                    nc.scalar.activation(e2, q_ps, AF.Exp)
                    r2 = small.tile([128, NT], F32, tag="r")
                    nc.scalar.activation(r2, q_ps, AF.Relu)
                    nc.vector.scalar_tensor_tensor(
                        qp_sb[:, c, ci * NT:(ci + 1) * NT], e2, 1.0, r2, OP.min, OP.add
                    )

        # block-diagonal kv lhsT (fp32r) and ksum-broadcast block-diagonal lhsT
        kvr_sb = persist.tile([128, 4, 128], F32R)
        ksb = persist.tile([128, 4, 128], F32R)
        for c in range(4):
            nc.vector.tensor_copy(kvr_sb[:, c, :], zeros_sb)
            nc.vector.tensor_copy(ksb[:, c, :], zeros_sb)
            nc.vector.tensor_copy(kvr_sb[0:64, c, 0:64], ar_sb[0:64, c, 0:HD])
            nc.vector.tensor_copy(kvr_sb[64:128, c, 64:128], ar_sb[64:128, c, 0:HD])
        for h in range(H):
            po = (h % 2) * 64
            c = h // 2
            nc.vector.tensor_scalar_mul(
                ksb[po:po + 64, c, po:po + 64],
                ones_col[po:po + 64, :],
                ar_sb[po:po + 64, c, HD:HD + 1],
            )

        # ---------------- phase 2: out = (q' kv) z; y = out.T Wout + b -----
        with tc.tile_pool(name="p2work", bufs=2) as work2, \
             tc.tile_pool(name="ps2", bufs=2, space="PSUM") as psum2:
            def opdn_chunk(ci):
                ost = work2.tile([128, 4, NT], F32R, tag="ost")
                for c in range(4):
                    op_ps = psum2.tile([128, NT], F32, tag="ops")
                    dn_ps = psum2.tile([128, NT], F32, tag="dns")
                    q_rhs = qp_sb[:, c, ci * NT:(ci + 1) * NT]
                    nc.tensor.matmul(op_ps, kvr_sb[:, c, :], q_rhs)
                    nc.tensor.matmul(dn_ps, ksb[:, c, :], q_rhs)
                    # z = 1/den via exp(-ln(den)); den is large & positive
                    lnz = small.tile([128, NT], F32, tag="lnz")
                    nc.scalar.activation(lnz, dn_ps, AF.Ln)
                    zb = small.tile([128, NT], F32, tag="zb")
                    nc.scalar.activation(zb, lnz, AF.Exp, scale=-1.0)
                    nc.vector.tensor_mul(ost[:, c, :], op_ps, zb)
                return ost

            def y_chunk(ci, ost):
                y_sb = work2.tile([128, 4, D], F32, tag="ysb")
                for t in range(4):
                    y_ps = psum2.tile([128, D], F32, tag="yps")
                    for c in range(4):
                        nc.tensor.matmul(
                            y_ps, ost[:, c, t * 128:(t + 1) * 128],
                            wout_sb[:, c, :], start=(c == 0), stop=(c == 3),
                        )
                    nc.vector.tensor_add(y_sb[:, t, :], y_ps, bias_sb)
                    if ci == NCH - 1:  # split the last store to cut the tail
                        yc = y_d[ci * NT + t * 128:ci * NT + (t + 1) * 128, :]
                        nc.sync.dma_start(out=yc.rearrange("(o p) f -> p o f", p=128),
                                          in_=y_sb[:, t:t + 1, :])
                if ci < NCH - 1:
                    yc = y_d[ci * NT:(ci + 1) * NT, :].rearrange("(t p) f -> p t f", p=128)
                    nc.sync.dma_start(out=yc, in_=y_sb)

            prev_o = (0, opdn_chunk(0))
            for ci in range(1, NCH):
                cur_o = (ci, opdn_chunk(ci))
                y_chunk(*prev_o)  # pipelined: hides chunk ci-1's z latency
                prev_o = cur_o
            y_chunk(*prev_o)


_CACHE = {}


def _get_nc():
    if "nc" in _CACHE:
        return _CACHE["nc"]
    nc = bacc.Bacc(trn_type="TRN2", num_devices=NCORES)
    xt_d = nc.dram_tensor("xt", [D, T], F32, kind="ExternalInput").ap()
    wqkv_d = nc.dram_tensor("wqkv", [128, 3, 4, D], F32, kind="ExternalInput").ap()
    wout_d = nc.dram_tensor("wout", [128, 4, D], F32, kind="ExternalInput").ap()
    bias_d = nc.dram_tensor("bias", [128, D], F32, kind="ExternalInput").ap()
    y_d = nc.dram_tensor("y", [T, D], F32, kind="ExternalOutput").ap()
    with tile.TileContext(nc) as tc:
        _build_kernel(tc, nc, xt_d, wqkv_d, wout_d, bias_d, y_d)
    nc.compile()
    _CACHE["nc"] = nc
    return nc


def kernel(x, Wqkv, Wout, bout, _trace=False, **_trace_kwargs):
    nc = _get_nc()
    x_flat = np.asarray(x, dtype=np.float32).reshape(B * N, D)
    # host-side packing to the SBUF layouts (cheap relayout, done per call)
    wq3 = np.asarray(Wqkv, dtype=np.float32).reshape(D, 3, D)  # [fi,{q,k,v},512]
    packed = np.empty((128, 3, 4, D), dtype=np.float32)
    for kc in range(4):
        packed[:, 0, kc, :] = wq3[kc * 128:(kc + 1) * 128, 1, :]  # k
        packed[:, 1, kc, :] = wq3[kc * 128:(kc + 1) * 128, 2, :]  # v
        packed[:, 2, kc, :] = wq3[kc * 128:(kc + 1) * 128, 0, :]  # q
    wqkv_p = np.ascontiguousarray(packed)
    wo = np.asarray(Wout, dtype=np.float32)
    wout_p = np.ascontiguousarray(wo.reshape(4, 128, D).transpose(1, 0, 2))
    bias_b = np.ascontiguousarray(
        np.broadcast_to(np.asarray(bout, dtype=np.float32).reshape(1, D), (128, D))
    )
    in_maps = []
    for c in range(NCORES):
        shard_t = np.ascontiguousarray(x_flat[c * T:(c + 1) * T].T)  # [512, 4096]
        in_maps.append({
            "xt": shard_t, "wqkv": wqkv_p, "wout": wout_p, "bias": bias_b,
        })
    res = run_bass_kernel_spmd(
        nc, in_maps, core_ids=list(range(NCORES)), trace=_trace, **_trace_kwargs
    )
    y = np.concatenate([res.results[c]["y"] for c in range(NCORES)], axis=0)
    out = y.reshape(B, N, D)
    if _trace:
        return out, res
    return out


# revision 22
# speedup vs baseline: 1.1460x; 1.1460x over previous
"""LinearSelfAttention (elu+1 linear attention) Trainium2 Bass kernel, v7.

Full inputs -> full output. Shards the 32768 tokens (B=4 x N=8192) across 8
NeuronCores as (batch, seq-half); the small kv / k-sum statistics are
all-reduced between the two cores sharing a batch. Weights are replicated.

Per-core structure (T=4096 tokens, 8 chunks of 512):
  - x arrives host-pre-transposed as x'^T [512 feat, 4096 tok] so
    feature-major tiles DMA straight into SBUF (no PE transposes).
  - Weights host-packed to SBUF layout; startup DMAs split per-kc and
    interleaved with the first x chunk so the first matmul starts early;
    a short burst of dummy matmuls warms the PE HAM clock while DMAs land.
  phase 1: k,v projections (fp32r, token-major), elu+1(k) (exp on ACT,
    relu+combine on DVE), per head-pair kv/ksum accumulation
    [k_2c|k_2c+1].T @ [v_2c |1| v_2c+1 |1] (N=130). The accum matmuls for
    chunk ci are emitted after chunk ci+1's k/v matmuls (software
    pipelining hides the elu latency).
  AllReduce of tight-packed stats [128,4,65] between seq-half pairs.
  q' projection (+elu, ACT-heavy on purpose) runs after the collective is
    triggered and hides its ~40us latency.
  phase 2: out'[e,n] via block-diag kv lhsT; denominator via ksum-broadcast
    block-diag lhsT (the matmul IS the partition broadcast - gpsimd
    partition_broadcast and any partition-offset custom-DVE input return
    garbage on HW); z = 1/den via one full-tile DVE reciprocal_approx_fast;
    ost = out'*z; y = ost.T @ Wout + bias (DVE add on the PSUM->SBUF move).
    y matmuls for chunk ci are emitted after chunk ci+1's out/den matmuls;
    the last chunk's store is split per t-tile to cut the tail.

All matmul operands are float32r (fp22-rounded fp32) - full PE rate for
N>=256, ~1e-4 relative error. fp32r matmuls require even N and outputs at
base partition 0. Ln/Exp for 1/x would thrash ACT table sets (walrus maps
exp->exp_and_others but ln->natural_log, so alternating Ln/Exp reloads
tables every op - 33 loads / 42us measured), hence the DVE reciprocal.
"""

import numpy as np

import concourse.bass as bass
import concourse.bacc as bacc
import concourse.mybir as mybir
import concourse.tile as tile
from concourse.bass_utils import run_bass_kernel_spmd

B, N, D, H, HD = 4, 8192, 512, 8, 64
NCORES = 8
T = B * N // NCORES          # 4096 tokens per core
NT = 512                     # tokens per chunk
NCH = T // NT                # 8 chunks
VW = 2 * HD + 2              # 130: [v_2c | 1 | v_2c+1 | 1]
SW = HD + 1                  # 65: [kv_h | ksum_h] packed width
F32 = mybir.dt.float32
F32R = mybir.dt.float32r
AF = mybir.ActivationFunctionType
OP = mybir.AluOpType

REPLICA_GROUPS = [[0, 1], [2, 3], [4, 5], [6, 7]]


def _r(ap):
    return ap.bitcast(F32R)


def _build_kernel(tc, nc, xt_d, wqkv_d, wout_d, bias_d, y_d):
    with (
        tc.tile_pool(name="consts", bufs=1) as consts,
        tc.tile_pool(name="persist", bufs=1) as persist,
        tc.tile_pool(name="small", bufs=2) as small,
        tc.tile_pool(name="dram", bufs=1, space="DRAM") as dram,
    ):
        xt_src = xt_d.rearrange("(kc p) t -> p kc t", p=128)

        # constants first: zeros/ones come from DVE only, no DMA needed
        scr_f32 = consts.tile([128, 128], F32)
        nc.vector.memset(scr_f32, 1.0)
        ones_col = consts.tile([128, HD], F32R)
        nc.vector.tensor_copy(ones_col, scr_f32[:, 0:HD])
        ones441 = consts.tile([128, 4, 1], F32R)
        nc.vector.tensor_copy(ones441, scr_f32[:, 0:4].rearrange("p (t o) -> p t o", o=1))
        zscr_f32 = consts.tile([128, 128], F32)
        nc.vector.memset(zscr_f32, 0.0)
        zeros_sb = consts.tile([128, 128], F32R)
        nc.vector.tensor_copy(zeros_sb, zscr_f32)

        wqkv_sb = consts.tile([128, 3, 4, D], F32R)
        wout_sb = consts.tile([128, 4, D], F32R)
        bias_sb = consts.tile([128, D], F32)

        with tc.tile_pool(name="p1work", bufs=2) as work, \
             tc.tile_pool(name="ps1", bufs=3, space="PSUM") as psum, \
             tc.tile_pool(name="psacc", bufs=2, space="PSUM") as psacc:
            # ~16 dummy matmuls (~3.4us) warm the PE HAM clock gate to 2.4GHz
            # while the first input DMAs are still in flight.
            warm_ps = psacc.tile([128, VW], F32, tag="acc", name="warm_ps")
            for _ in range(16):
                nc.tensor.matmul(warm_ps[:, 0:128], zeros_sb, zeros_sb)
            nc.vector.tensor_copy(zscr_f32[0:1, 0:2], warm_ps[0:1, 0:2])

            # first x chunk's t-slices interleaved with the per-kc k-weight
            # slices: the first k matmul is gated on only ~0.5 MB of DMA.
            xt0 = work.tile([128, 4, NT], F32R, tag="xt")
            for t in range(4):
                nc.sync.dma_start(
                    out=xt0[:, :, t * 128:(t + 1) * 128],
                    in_=_r(xt_src[:, :, t * 128:(t + 1) * 128]),
                )
                nc.sync.dma_start(out=wqkv_sb[:, 0, t, :], in_=_r(wqkv_d[:, 0, t, :]))
            for kc in range(4):
                nc.sync.dma_start(out=wqkv_sb[:, 1, kc, :], in_=_r(wqkv_d[:, 1, kc, :]))

            # q'+ (elu(q)+1), feature-major, persistent: [fo, n]
            qp_sb = persist.tile([128, 4, T], F32R)
            cc_sb = persist.tile([128, 4, VW], F32)
            nc.vector.memset(cc_sb, 0.0)

            # ---------------- phase 1: k,v -> kv/ksum stats ----------------
            def kv_chunk(ci, xt_sb):
                """k/v projection + elu(k) for chunk ci; returns (kp, v_sb)."""
                v_sb = work.tile([128, 4, 4 * VW], F32R, tag="vsb")
                for c in range(4):
                    for u in range(2):
                        col = c * VW + HD + u * (HD + 1)
                        nc.vector.tensor_copy(v_sb[:, :, col:col + 1], ones441)
                kp = work.tile([128, 4, D], F32R, tag="kp")
                for t in range(4):
                    k_ps = psum.tile([128, D], F32, tag="kps")
                    v_ps = psum.tile([128, D], F32, tag="vps")
                    for kc in range(4):
                        st, sp = kc == 0, kc == 3
                        lhsT = xt_sb[:, kc, t * 128:(t + 1) * 128]
                        nc.tensor.matmul(k_ps, lhsT, wqkv_sb[:, 0, kc, :], start=st, stop=sp)
                        nc.tensor.matmul(v_ps, lhsT, wqkv_sb[:, 1, kc, :], start=st, stop=sp)
                    nc.vector.tensor_copy(
                        v_sb[:, t, :].rearrange("p (c u e) -> p c u e", c=4, u=2)[:, :, :, 0:HD],
                        v_ps.rearrange("p (c u e) -> p c u e", c=4, u=2),
                    )
                    # elu(k)+1 = min(exp(k),1) + relu(k); exp on ACT, rest DVE
                    e_sb = small.tile([128, D], F32, tag="e")
                    nc.scalar.activation(e_sb, k_ps, AF.Exp)
                    r_sb = small.tile([128, D], F32, tag="r")
                    nc.vector.tensor_scalar_max(r_sb, k_ps, 0.0)
                    nc.vector.scalar_tensor_tensor(kp[:, t, :], e_sb, 1.0, r_sb, OP.min, OP.add)
                return kp, v_sb

            def kv_accum(kp, v_sb):
                for c in range(4):
                    acc_ps = psacc.tile([128, VW], F32, tag="acc")
                    for t in range(4):
                        nc.tensor.matmul(
                            acc_ps,
                            kp[:, t, c * 128:(c + 1) * 128],
                            v_sb[:, t, c * VW:(c + 1) * VW],
                            start=(t == 0), stop=(t == 3),
                        )
                    nc.vector.tensor_add(cc_sb[:, c, :], cc_sb[:, c, :], acc_ps)

            prev = kv_chunk(0, xt0)
            for ci in range(1, NCH):
                xt_sb = work.tile([128, 4, NT], F32R, tag="xt")
                nc.sync.dma_start(out=xt_sb, in_=_r(xt_src[:, :, ci * NT:(ci + 1) * NT]))
                if ci == 1:  # q/wout weights aren't needed until much later;
                    nc.sync.dma_start(out=wqkv_sb[:, 2], in_=_r(wqkv_d[:, 2]))
                if ci == 2:  # keep them behind the early x chunks in the queue
                    nc.sync.dma_start(out=wout_sb, in_=_r(wout_d))
                    nc.sync.dma_start(out=bias_sb, in_=bias_d)
                cur = kv_chunk(ci, xt_sb)
                kv_accum(*prev)  # pipelined: hides chunk ci-1's elu latency
                prev = cur
            kv_accum(*prev)

        # lhsT skeletons for phase 2: zero-fill now (no collective dep), so
        # only the small ar-dependent writes sit on the post-collective path
        kvr_sb = persist.tile([128, 4, 128], F32R)
        ksb_sb = persist.tile([128, 4, 128], F32R)
        for c in range(4):
            nc.vector.tensor_copy(kvr_sb[:, c, :], zeros_sb)
            nc.vector.tensor_copy(ksb_sb[:, c, :], zeros_sb)

        # ---------------- all-reduce kv/ksum between seq-half pairs --------
        # tight-pack [128,4,130] -> [128,4,65]: rows 0:64 hold [kv_2c|ksum],
        # rows 64:128 hold [kv_2c+1|ksum] (halves the collective payload).
        with tc.tile_pool(name="qwork", bufs=2) as qwork, \
             tc.tile_pool(name="work2", bufs=2) as work2, \
             tc.tile_pool(name="ps2", bufs=2, space="PSUM") as psum2:
            # prefetch the first two q'-pass x chunks so the PE rolls straight
            # from phase 1 into the q' projection
            xtq_pre = []
            for ci in range(2):
                xt_sb = qwork.tile([128, 4, NT], F32R, tag="xtq")
                nc.sync.dma_start(out=xt_sb, in_=_r(xt_src[:, :, ci * NT:(ci + 1) * NT]))
                xtq_pre.append(xt_sb)

            cc_tx = persist.tile([128, 4, SW], F32)
            for c in range(4):
                nc.vector.tensor_copy(cc_tx[0:64, c, :], cc_sb[0:64, c, 0:SW])
                nc.vector.tensor_copy(cc_tx[64:128, c, :], cc_sb[64:128, c, SW:2 * SW])
            cc_in = dram.tile([128, 4, SW], F32)
            cc_out = dram.tile([128, 4, SW], F32)
            nc.sync.dma_start(out=cc_in, in_=cc_tx)
            nc.gpsimd.collective_compute(
                "AllReduce", OP.add,
                replica_groups=REPLICA_GROUPS,
                ins=[cc_in.opt()], outs=[cc_out.opt()],
            )
            ar_sb = persist.tile([128, 4, SW], F32)
            nc.sync.dma_start(out=ar_sb, in_=cc_out)

            # ------------- q' projection (overlaps the collective) ----------
            for ci in range(NCH):
                if ci < 2:
                    xt_sb = xtq_pre[ci]
                else:
                    xt_sb = qwork.tile([128, 4, NT], F32R, tag="xtq")
                    nc.sync.dma_start(out=xt_sb, in_=_r(xt_src[:, :, ci * NT:(ci + 1) * NT]))
                for c in range(4):
                    q_ps = psum2.tile([128, NT], F32, tag="qps")
                    for kc in range(4):
                        nc.tensor.matmul(
                            q_ps,
                            wqkv_sb[:, 2, kc, c * 128:(c + 1) * 128],
                            xt_sb[:, kc, :],
                            start=(kc == 0), stop=(kc == 3),
                        )
                    e2 = small.tile([128, NT], F32, tag="e")
                    nc.scalar.activation(e2, q_ps, AF.Exp)
                    r2 = small.tile([128, NT], F32, tag="r")
                    nc.scalar.activation(r2, q_ps, AF.Relu)
                    nc.vector.scalar_tensor_tensor(
                        qp_sb[:, c, ci * NT:(ci + 1) * NT], e2, 1.0, r2, OP.min, OP.add
                    )

            # ar-dependent lhsT fills: block-diag kv and ksum broadcast (the
            # matmul with 64 identical lhsT columns IS the z broadcast)
            for c in range(4):
                nc.vector.tensor_copy(kvr_sb[0:64, c, 0:64], ar_sb[0:64, c, 0:HD])
                nc.vector.tensor_copy(kvr_sb[64:128, c, 64:128], ar_sb[64:128, c, 0:HD])
            for h in range(H):
                po = (h % 2) * 64
                c = h // 2
                nc.vector.tensor_scalar_mul(
                    ksb_sb[po:po + 64, c, po:po + 64],
                    ones_col[po:po + 64, :],
                    ar_sb[po:po + 64, c, HD:HD + 1],
                )

            # ------------- phase 2: out = (q' kv) z; y = out.T Wout + b -----
            def opdn_chunk(ci):
                ost = work2.tile([128, 4, NT], F32R, tag="ost")
                for c in range(4):
                    op_ps = psum2.tile([128, NT], F32, tag="ops")
                    dn_ps = psum2.tile([128, NT], F32, tag="dns")
                    q_rhs = qp_sb[:, c, ci * NT:(ci + 1) * NT]
                    nc.tensor.matmul(op_ps, kvr_sb[:, c, :], q_rhs)
                    nc.tensor.matmul(dn_ps, ksb_sb[:, c, :], q_rhs)
                    # z = 1/den (den is large & positive; ~18-bit approx ok)
                    zb = small.tile([128, NT], F32, tag="zb")
                    nc.vector.reciprocal_approx_fast(zb, dn_ps)
                    nc.vector.tensor_mul(ost[:, c, :], op_ps, zb)
                return ost

            def y_chunk(ci, ost):
                y_sb = work2.tile([128, 4, D], F32, tag="ysb")
                for t in range(4):
                    y_ps = psum2.tile([128, D], F32, tag="yps")
                    for c in range(4):
                        nc.tensor.matmul(
                            y_ps, ost[:, c, t * 128:(t + 1) * 128],
                            wout_sb[:, c, :], start=(c == 0), stop=(c == 3),
                        )
                    nc.vector.tensor_add(y_sb[:, t, :], y_ps, bias_sb)
                    if ci == NCH - 1:  # split the last store to cut the tail
                        yc = y_d[ci * NT + t * 128:ci * NT + (t + 1) * 128, :]
                        nc.sync.dma_start(out=yc.rearrange("(o p) f -> p o f", p=128),
                                          in_=y_sb[:, t:t + 1, :])
                if ci < NCH - 1:
                    yc = y_d[ci * NT:(ci + 1) * NT, :].rearrange("(t p) f -> p t f", p=128)
                    nc.sync.dma_start(out=yc, in_=y_sb)

            prev_o = (0, opdn_chunk(0))
            for ci in range(1, NCH):
                cur_o = (ci, opdn_chunk(ci))
                y_chunk(*prev_o)  # pipelined: hides chunk ci-1's z latency
                prev_o = cur_o
            y_chunk(*prev_o)


_CACHE = {}


def _get_nc():
    if "nc" in _CACHE:
        return _CACHE["nc"]
    nc = bacc.Bacc(trn_type="TRN2", num_devices=NCORES)
    xt_d = nc.dram_tensor("xt", [D, T], F32, kind="ExternalInput").ap()
    wqkv_d = nc.dram_tensor("wqkv", [128, 3, 4, D], F32, kind="ExternalInput").ap()
    wout_d = nc.dram_tensor("wout", [128, 4, D], F32, kind="ExternalInput").ap()
    bias_d = nc.dram_tensor("bias", [128, D], F32, kind="ExternalInput").ap()
    y_d = nc.dram_tensor("y", [T, D], F32, kind="ExternalOutput").ap()
    with tile.TileContext(nc) as tc:
        _build_kernel(tc, nc, xt_d, wqkv_d, wout_d, bias_d, y_d)
    nc.compile()
    _CACHE["nc"] = nc
    return nc


def kernel(x, Wqkv, Wout, bout, _trace=False, **_trace_kwargs):
    nc = _get_nc()
    x_flat = np.asarray(x, dtype=np.float32).reshape(B * N, D)
    # host-side packing to the SBUF layouts (cheap relayout, done per call)
    wq3 = np.asarray(Wqkv, dtype=np.float32).reshape(D, 3, D)  # [fi,{q,k,v},512]
    packed = np.empty((128, 3, 4, D), dtype=np.float32)
    for kc in range(4):
        packed[:, 0, kc, :] = wq3[kc * 128:(kc + 1) * 128, 1, :]  # k
        packed[:, 1, kc, :] = wq3[kc * 128:(kc + 1) * 128, 2, :]  # v
        packed[:, 2, kc, :] = wq3[kc * 128:(kc + 1) * 128, 0, :]  # q
    wqkv_p = np.ascontiguousarray(packed)
    wo = np.asarray(Wout, dtype=np.float32)
    wout_p = np.ascontiguousarray(wo.reshape(4, 128, D).transpose(1, 0, 2))
    bias_b = np.ascontiguousarray(
        np.broadcast_to(np.asarray(bout, dtype=np.float32).reshape(1, D), (128, D))
    )
    in_maps = []
    for c in range(NCORES):
        shard_t = np.ascontiguousarray(x_flat[c * T:(c + 1) * T].T)  # [512, 4096]
        in_maps.append({
            "xt": shard_t, "wqkv": wqkv_p, "wout": wout_p, "bias": bias_b,
        })
    res = run_bass_kernel_spmd(
        nc, in_maps, core_ids=list(range(NCORES)), trace=_trace, **_trace_kwargs
    )
    y = np.concatenate([res.results[c]["y"] for c in range(NCORES)], axis=0)
    out = y.reshape(B, N, D)
    if _trace:
        return out, res
    return out
